# revision 4
# baseline (speedup 1.0000x reference)
"""DeepseekV4-style attention (partial-RoPE LoRA-Q GQA sliding-window) on 8
Trainium2 NeuronCores.

Sharding: core c = 4*b + g handles batch b (of 2) and GQA group g (of 4):
q heads 4g..4g+3, kv head g, the matching column slices of Wq/Wk/Wv and row
slice of Wo.  Each core computes a partial output; the host sums the four
group partials per batch.

Perf notes vs v1:
- all matmul operands bf16 (rel tol is 2e-2; measured err ~1e-3), halving
  DMA bytes and enabling 1024-wide moving operands in the QKV projection.
- Wq_eff = Wqa @ Wqb is precomputed on the host, deleting the second
  projection stage entirely.
- QKV projection loops output-major (v, k, q0..q3), each output accumulating
  one PSUM buffer over all 16 k-tiles: no PSUM-reuse stalls, 2 banks live.
- attention and output projection are fused per 512-row q-block so out-DMAs
  spread across the whole phase; QK runs 2 kt ahead of PV; softmax epilogue
  uses reciprocal_approx_fast (pd is PSUM, so no 2-PSUM-operand divide).
- weights land as one packed DMA each, issued before everything else.
"""

import numpy as np
import ml_dtypes
import concourse.bass as bass
import concourse.mybir as mybir
import concourse.tile as tile
from concourse.bass_utils import run_bass_kernel_spmd

F32 = mybir.dt.float32
BF16 = mybir.dt.bfloat16
ACTF = mybir.ActivationFunctionType
ALU = mybir.AluOpType

B, S, D = 2, 2048, 2048
H, KVH, HD = 16, 4, 128
ROT, LORA, WINDOW = 64, 512, 1024
ROPE_BASE = 10000.0
SCALE = HD ** -0.5

HPC = H // KVH          # 4 q heads per core
SB = 512                # q-block for attention + out projection
SBL = 512               # seq block for the QKV projection
NSB = S // SB           # 4
NBL = S // SBL          # 4
KT = D // 128           # 16 contraction tiles over D
N_CORES = 8
NPB = ml_dtypes.bfloat16


def _split_multiwaits(nc):
    """This image's walrus accepts only one embedded SyncWait per instruction;
    split Tile's multi-wait sync_infos into standalone event-semaphore waits."""
    n = 0
    for func in nc.m.functions:
        for bb in func.blocks:
            insts = list(bb.instructions)
            out = []
            changed = False
            for inst in insts:
                si = inst.sync_info
                if si is not None and si.on_wait and len(si.on_wait) > 1:
                    waits = list(si.on_wait)
                    for w in waits[:-1]:
                        ev = mybir.InstEventSemaphore(
                            name=f"{inst.name}_wsplit_{n}", ins=[], outs=[]
                        )
                        ev.engine = inst.engine
                        ev.sync_info = mybir.SyncInfo(on_wait=[w], on_update=[])
                        out.append(ev)
                        n += 1
                    inst.sync_info = mybir.SyncInfo(
                        on_wait=[waits[-1]], on_update=list(si.on_update or [])
                    )
                    changed = True
                out.append(inst)
            if changed:
                bb.instructions = out
    return n


def build_nc():
    nc = bass.Bass()
    hid = nc.dram_tensor("hid", [128, KT, S], BF16, kind="ExternalInput")
    wq = nc.dram_tensor("wq", [128, KT, LORA], BF16, kind="ExternalInput")
    wkv = nc.dram_tensor("wkv", [128, KT, 256], BF16, kind="ExternalInput")
    wo = nc.dram_tensor("wo", [128, HPC, D], BF16, kind="ExternalInput")
    rcs = nc.dram_tensor("rcs", [128, S], BF16, kind="ExternalInput")
    out = nc.dram_tensor("out", [S, D], F32, kind="ExternalOutput")

    with tile.TileContext(nc) as tc:
        with (
            tc.tile_pool(name="cst", bufs=1) as cst,
            tc.tile_pool(name="big", bufs=1) as big,
        ):
            # ---- persistent tiles ----
            wq_sb = big.tile([128, KT, LORA], BF16, tag="wq_sb")
            wkv_sb = big.tile([128, KT, 256], BF16, tag="wkv_sb")
            wo_sb = big.tile([128, HPC, D], BF16, tag="wo_sb")
            hs = big.tile([128, KT, S], BF16, tag="hs")
            qT = big.tile([128, HPC * S], BF16, tag="qT")   # per-head Q^T [hd, s]
            kT = big.tile([128, S], BF16, tag="kT")
            vT = big.tile([128, S], BF16, tag="vT")
            vnat = big.tile([128, S], BF16, tag="vnat")     # V rows, chunk t at cols t*128
            ropeC = cst.tile([64, S], BF16, tag="ropeC")
            ropeS = cst.tile([64, S], BF16, tag="ropeS")

            # ---- startup DMAs, ordered by first use ----
            nc.sync.dma_start(out=wkv_sb[:], in_=wkv[:])
            for kc in range(0, KT, 4):   # first hid block in 4 chunks
                nc.sync.dma_start(
                    out=hs[:, kc:kc + 4, 0:SBL], in_=hid[:, kc:kc + 4, 0:SBL]
                )
            nc.sync.dma_start(out=wq_sb[:], in_=wq[:])
            nc.sync.dma_start(out=ropeC[:], in_=rcs[0:64, :])
            nc.sync.dma_start(out=ropeS[:], in_=rcs[64:128, :])
            for blk in range(1, NBL):
                nc.sync.dma_start(
                    out=hs[:, :, blk * SBL:(blk + 1) * SBL],
                    in_=hid[:, :, blk * SBL:(blk + 1) * SBL],
                )
            nc.sync.dma_start(out=wo_sb[:], in_=wo[:])

            onesf = cst.tile([128, 128], F32, tag="onesf")
            nc.vector.memset(onesf[:], 1.0)
            ones = cst.tile([128, 128], BF16, tag="ones")
            nc.vector.tensor_copy(ones[:], onesf[:])
            identf = cst.tile([128, 128], F32, tag="identf")
            nc.gpsimd.affine_select(
                out=identf[:], in_=onesf[:], pattern=[[1, 128]],
                compare_op=ALU.is_equal, fill=0.0, base=0, channel_multiplier=-1,
            )
            ident = cst.tile([128, 128], BF16, tag="ident")
            nc.vector.tensor_copy(ident[:], identf[:])

            def rope_apply(dst, sl, rsl, rp, w):
                # dst rows 0:64 hold [x1; x2]; rotate in place (T-layout).
                # DVE ops need equal SBUF base partitions, so the half-swap
                # goes through a small SBUF->SBUF DMA.
                swp = rp.tile([64, w], BF16, tag="swp")
                nc.sync.dma_start(out=swp[0:32, :], in_=dst[32:64, sl])
                nc.sync.dma_start(out=swp[32:64, :], in_=dst[0:32, sl])
                csb = rp.tile([64, w], BF16, tag="csb")
                nc.vector.tensor_mul(csb[:], dst[0:64, sl], ropeC[:, rsl])
                tsin = rp.tile([64, w], BF16, tag="tsin")
                nc.vector.tensor_mul(tsin[:], swp[:], ropeS[:, rsl])
                nc.vector.tensor_sub(dst[0:32, sl], csb[0:32, :], tsin[0:32, :])
                nc.vector.tensor_add(dst[32:64, sl], csb[32:64, :], tsin[32:64, :])

            # ---- phase A: q^T (4 heads), k^T, v^T, v natural ----
            with (
                tc.tile_pool(name="rp", bufs=2) as rp,
                tc.tile_pool(name="psA", bufs=2, space="PSUM") as psA,
                tc.tile_pool(name="psT", bufs=2, space="PSUM") as psT,
            ):
                for blk in range(NBL):
                    bsl = slice(blk * SBL, (blk + 1) * SBL)

                    def proj(stat_of_k, dst, dsl, name):
                        p = psA.tile([128, SBL], F32, tag="p", name=f"p_{name}_{blk}")
                        for k in range(KT):
                            nc.tensor.matmul(
                                p[:], stat_of_k(k), hs[:, k, bsl],
                                start=(k == 0), stop=(k == KT - 1),
                            )
                        nc.scalar.copy(dst[:, dsl], p[:])

                    proj(lambda k: wkv_sb[:, k, 128:256], vT, bsl, "v")
                    proj(lambda k: wkv_sb[:, k, 0:128], kT, bsl, "k")
                    # V natural: PE-transpose the 4 128-chunks of this block
                    # (vT copy completes during the k pass)
                    for t in range(blk * 4, blk * 4 + 4):
                        tp = psT.tile([128, 128], BF16, tag="tp")
                        nc.tensor.transpose(tp[:], vT[:, t * 128:(t + 1) * 128], ident[:])
                        nc.vector.tensor_copy(vnat[:, t * 128:(t + 1) * 128], tp[:])
                    rope_apply(kT, bsl, bsl, rp, SBL)
                    for m in range(HPC):
                        dsl = slice(m * S + blk * SBL, m * S + (blk + 1) * SBL)
                        proj(lambda k: wq_sb[:, k, m * 128:(m + 1) * 128], qT, dsl,
                             f"q{m}")
                        rope_apply(qT, dsl, bsl, rp, SBL)

            # ---- phase B: attention + output projection, fused per q-block ----
            with (
                tc.tile_pool(name="atn", bufs=1) as atn,
                tc.tile_pool(name="ex", bufs=4) as ex,
                tc.tile_pool(name="rc", bufs=2) as rc,
                tc.tile_pool(name="od", bufs=3) as od,
                tc.tile_pool(name="psL", bufs=3, space="PSUM") as psL,
                tc.tile_pool(name="psO", bufs=2, space="PSUM") as psO,
                tc.tile_pool(name="psD", bufs=1, space="PSUM") as psD,
                tc.tile_pool(name="psW", bufs=2, space="PSUM") as psW,
            ):
                attnB = atn.tile([128, HPC * SB], BF16, tag="attnB")
                for qb in range(NSB):
                    q0 = qb * SB
                    kt_lo = max(0, q0 - WINDOW + 1) // 128
                    kt_hi = q0 // 128 + 3
                    nkt = kt_hi - kt_lo + 1
                    for h in range(HPC):
                        qsl = slice(h * S + q0, h * S + q0 + SB)
                        po = psO.tile([128, SB], F32, tag="po")
                        pd = psD.tile([128, SB], F32, tag="pd")
                        pls = {}

                        def qk(kt):
                            pl = psL.tile([128, SB], F32, tag="pl",
                                          name=f"pl_{qb}_{h}_{kt}")
                            nc.tensor.matmul(
                                pl[:], kT[:, kt * 128:(kt + 1) * 128], qT[:, qsl],
                                start=True, stop=True,
                            )
                            pls[kt] = pl

                        qk(kt_lo)
                        if nkt > 1:
                            qk(kt_lo + 1)
                        for i, kt in enumerate(range(kt_lo, kt_hi + 1)):
                            if kt + 2 <= kt_hi:
                                qk(kt + 2)
                            pl = pls.pop(kt)
                            dp = kt * 128 - q0
                            e = ex.tile([128, SB], BF16, tag="e")
                            nc.scalar.activation(e[:], pl[:], ACTF.Exp, scale=SCALE)
                            if dp >= 0:
                                # causal edge: keep j - i - dp >= 0
                                nc.gpsimd.affine_select(
                                    out=e[:], in_=e[:], pattern=[[1, SB]],
                                    compare_op=ALU.is_ge, fill=0.0,
                                    base=-dp, channel_multiplier=-1,
                                )
                            elif dp <= SB - WINDOW:
                                # window edge: keep (q0+j)-(k0+i) < WINDOW
                                nc.gpsimd.affine_select(
                                    out=e[:], in_=e[:], pattern=[[-1, SB]],
                                    compare_op=ALU.is_ge, fill=0.0,
                                    base=WINDOW - 1 + dp, channel_multiplier=1,
                                )
                            st, sp = (i == 0), (kt == kt_hi)
                            nc.tensor.matmul(
                                po[:], vnat[:, kt * 128:(kt + 1) * 128], e[:],
                                start=st, stop=sp,
                            )
                            nc.tensor.matmul(pd[:], ones[:], e[:], start=st, stop=sp)
                        rec = rc.tile([128, SB], F32, tag="rec")
                        nc.vector.reciprocal(rec[:], pd[:])
                        nc.vector.tensor_mul(
                            attnB[:, h * SB:(h + 1) * SB], po[:], rec[:]
                        )
                    # output projection for this q-block's 4 row-chunks
                    for t in range(4):
                        asl = slice(qb * SB + t * 128, qb * SB + (t + 1) * 128)
                        for n in range(4):
                            pw = psW.tile([128, SB], F32, tag="pw")
                            for h in range(HPC):
                                nc.tensor.matmul(
                                    pw[:],
                                    attnB[:, h * SB + t * 128: h * SB + (t + 1) * 128],
                                    wo_sb[:, h, n * SB:(n + 1) * SB],
                                    start=(h == 0), stop=(h == HPC - 1),
                                )
                            ot = od.tile([128, SB], F32, tag="ot")
                            if n % 2 == 0:
                                nc.scalar.copy(ot[:], pw[:])
                            else:
                                nc.vector.tensor_copy(ot[:], pw[:])
                            nc.sync.dma_start(
                                out=out[asl, n * SB:(n + 1) * SB], in_=ot[:]
                            )
    _split_multiwaits(nc)
    return nc


_NC = None


def _get_nc():
    global _NC
    if _NC is None:
        _NC = build_nc()
    return _NC


def _pack_k128(a):
    # [D, N] -> [128, D//128, N]
    d, n = a.shape
    return np.ascontiguousarray(
        a.reshape(d // 128, 128, n).transpose(1, 0, 2).astype(NPB)
    )


def _make_in_maps(hidden, position_ids, Wqa, Wqb, Wk, Wv, Wo):
    hidden = np.asarray(hidden, dtype=np.float32)
    position_ids = np.asarray(position_ids)
    Wqa = np.asarray(Wqa, dtype=np.float32)
    Wqb = np.asarray(Wqb, dtype=np.float32)
    Wk = np.asarray(Wk, dtype=np.float32)
    Wv = np.asarray(Wv, dtype=np.float32)
    Wo = np.asarray(Wo, dtype=np.float32)

    inv_freq = 1.0 / (ROPE_BASE ** (np.arange(0, ROT, 2, dtype=np.float32) / ROT))
    hidp = [_pack_k128(np.ascontiguousarray(hidden[b].T)) for b in range(B)]
    in_maps = []
    for c in range(N_CORES):
        b, g = c // KVH, c % KVH
        pos = position_ids[b].astype(np.float32)
        freqs = pos[:, None] * inv_freq[None, :]        # [S, 32]
        cosT = np.cos(freqs).T.astype(np.float32)       # [32, S]
        sinT = np.sin(freqs).T.astype(np.float32)
        rcs = np.concatenate([cosT, cosT, sinT, sinT], axis=0)  # [128, S]
        wq_eff = Wqa @ Wqb[:, g * HPC * HD:(g + 1) * HPC * HD]  # [D, 512] f32
        wkv = np.concatenate(
            [Wk[:, g * HD:(g + 1) * HD], Wv[:, g * HD:(g + 1) * HD]], axis=1
        )
        in_maps.append({
            "hid": hidp[b],
            "wq": _pack_k128(wq_eff),
            "wkv": _pack_k128(wkv),
            "wo": np.ascontiguousarray(
                Wo[g * HPC * HD:(g + 1) * HPC * HD, :]
                .reshape(HPC, 128, D).transpose(1, 0, 2).astype(NPB)
            ),
            "rcs": rcs.astype(NPB),
        })
    return in_maps


def _run(inputs, trace=False):
    nc = _get_nc()
    in_maps = _make_in_maps(**inputs)
    res = run_bass_kernel_spmd(nc, in_maps, list(range(N_CORES)), trace=trace)
    out = np.zeros((B, S, D), dtype=np.float32)
    for c in range(N_CORES):
        out[c // KVH] += res.results[c]["out"]
    return out, res


def kernel(**inputs) -> np.ndarray:
    return _run(inputs, trace=False)[0]
